# revision 10
# baseline (speedup 1.0000x reference)
"""Iteration-2 reconstruction (73170 ns measured): fp16 I/O, f32r carry,
2-way sel pairs via tile_position row groups, flush after mains, outputs
per-group on the GpSimd ring. See kernel_iter4.py for the newer variant."""

import numpy as np

import concourse.bacc as bacc
import concourse.tile as tile
from concourse import mybir
from concourse.bass_utils import run_bass_kernel_spmd

B, T, F = 8, 4096, 1024
P = 128
NBLK = T // P  # 32
FH = 512       # one PSUM bank of f32
NHALF = F // FH
CPG = 2        # blocks per pipeline stage

F16 = mybir.dt.float16
F32 = mybir.dt.float32
F32R = mybir.dt.float32r


def _build():
    nc = bacc.Bacc(None, target_bir_lowering=False)
    x_dram = nc.dram_tensor("x", [T, F], F16, kind="ExternalInput")
    out_dram = nc.dram_tensor("out", [T, F], F16, kind="ExternalOutput")

    lt_np = np.triu(np.ones((P, P), dtype=np.float16))  # lt[s,t]=1 for s<=t
    sel_np = np.zeros((64, P), dtype=np.float32)        # row-group selectors
    sel_np[31, :] = 1.0
    sel_np[63, :] = 1.0
    recip_np = np.ascontiguousarray(
        (1.0 / (np.arange(1, T + 1, dtype=np.float64))).astype(np.float32)
        .reshape(NBLK, P).T
    )  # [p, i] = 1/(i*128+p+1)
    lt_dram = nc.inline_tensor(lt_np, "lt_const")
    sel_dram = nc.inline_tensor(sel_np, "sel_const")
    recip_dram = nc.inline_tensor(recip_np, "recip_const")

    x_rot = x_dram.rearrange("(n p) f -> p n f", p=P)
    out_rot = out_dram.rearrange("(n p) f -> p n f", p=P)

    with tile.TileContext(nc) as tc:
        with (
            tc.tile_pool(name="const", bufs=1) as cpool,
            tc.tile_pool(name="xin", bufs=6) as xpool,
            tc.tile_pool(name="xout", bufs=3) as opool,
            tc.tile_pool(name="run", bufs=6) as rpool,
            tc.tile_pool(name="psum", bufs=4, space="PSUM") as ppool,
        ):
            lt = cpool.tile([P, P], F16)
            nc.gpsimd.dma_start(lt[:], lt_dram[:])
            sel_f32 = cpool.tile([64, P], F32)
            nc.gpsimd.dma_start(sel_f32[:], sel_dram[:])
            sel = cpool.tile([64, P], F32R)
            nc.vector.tensor_copy(sel[:], sel_f32[:])
            recip = cpool.tile([P, NBLK], F32)
            nc.gpsimd.dma_start(recip[:], recip_dram[:])

            def flush(pend, last=False):
                psums, carries, pbase, pgsz = pend
                ot = opool.tile([P, CPG, F], F16, tag="ot")
                for c in range(pgsz):
                    if carries[c] is not None:
                        for h in range(NHALF):
                            hs = slice(h * FH, (h + 1) * FH)
                            rs = slice(32 * h, 32 * h + 32)
                            nc.tensor.matmul(
                                psums[c][:, hs], sel[rs, :], carries[c][rs, :],
                                start=False, stop=True,
                                tile_position=(32 * h, 0),
                            )
                if last:
                    # Drain: final two scales on ScalarE and VectorE
                    # concurrently (the carry chain is done by now), each
                    # followed by its own output DMA.
                    nc.scalar.activation(
                        ot[:, 0, :], psums[0][:],
                        mybir.ActivationFunctionType.Identity,
                        scale=recip[:, pbase : pbase + 1],
                    )
                    nc.gpsimd.dma_start(
                        out_rot[:, pbase : pbase + 1, :], ot[:, 0:1, :]
                    )
                    nc.vector.tensor_scalar(
                        ot[:, 1, :], psums[1][:],
                        recip[:, pbase + 1 : pbase + 2], None,
                        mybir.AluOpType.mult,
                    )
                    nc.gpsimd.dma_start(
                        out_rot[:, pbase + 1 : pbase + 2, :], ot[:, 1:2, :]
                    )
                else:
                    for c in range(pgsz):
                        i = pbase + c
                        nc.scalar.activation(
                            ot[:, c, :], psums[c][:],
                            mybir.ActivationFunctionType.Identity,
                            scale=recip[:, i : i + 1],
                        )
                    nc.gpsimd.dma_start(
                        out_rot[:, pbase : pbase + pgsz, :], ot[:, 0:pgsz, :]
                    )

            carry = None  # [64, FH] f32r split rows, see docstring
            pend = None
            base = 0
            for g in range(NBLK // CPG):
                if g == 0:
                    # Ramp: 128 KiB F-half DMAs so main(0, h0) starts as
                    # early as possible.
                    xt = xpool.tile([P, CPG, F], F16, tag="xt")
                    for c in range(CPG):
                        for h in range(NHALF):
                            hs = slice(h * FH, (h + 1) * FH)
                            nc.sync.dma_start(
                                xt[:, c : c + 1, hs], x_rot[:, c : c + 1, hs]
                            )
                else:
                    xt = xpool.tile([P, CPG, F], F16, tag="xt")
                    nc.sync.dma_start(xt[:], x_rot[:, base : base + CPG, :])

                psums = []
                carries = []
                for c in range(CPG):
                    i = base + c
                    ps = ppool.tile([P, F], F32)
                    psums.append(ps)
                    carries.append(carry)
                    for h in range(NHALF):
                        hs = slice(h * FH, (h + 1) * FH)
                        nc.tensor.matmul(
                            ps[:, hs], lt[:], xt[:, c, hs],
                            start=True, stop=(i == 0),
                        )
                    if i < NBLK - 1:
                        new_carry = rpool.tile([64, FH], F32R)
                        for h in range(NHALF):
                            hs = slice(h * FH, (h + 1) * FH)
                            rs = slice(32 * h, 32 * h + 32)
                            if carry is None:
                                nc.vector.tensor_copy(
                                    new_carry[rs, :], ps[96:P, hs]
                                )
                            else:
                                nc.vector.tensor_tensor(
                                    new_carry[rs, :],
                                    carry[rs, :].bitcast(F32),
                                    ps[96:P, hs],
                                    mybir.AluOpType.add,
                                )
                        carry = new_carry

                if pend is not None:
                    flush(pend)
                pend = (psums, carries, base, CPG)
                base += CPG

            flush(pend, last=True)

    nc.compile()
    return nc


_NC_CACHE = None
last_results = None  # BassKernelResults of the most recent run (for test harness)


def kernel(inputs: np.ndarray) -> np.ndarray:
    global _NC_CACHE, last_results
    if _NC_CACHE is None:
        _NC_CACHE = _build()
    nc = _NC_CACHE
    x = np.asarray(inputs)
    assert x.shape == (B, T, F), x.shape
    x16 = np.ascontiguousarray(x.astype(np.float16))
    in_maps = [{"x": x16[b]} for b in range(B)]
    res = run_bass_kernel_spmd(nc, in_maps, core_ids=list(range(B)))
    last_results = res
    return np.stack([r["out"] for r in res.results], axis=0).astype(np.float32)


# revision 11
# speedup vs baseline: 1.2451x; 1.2451x over previous
"""Iteration-2 reconstruction (73170 ns measured): fp16 I/O, f32r carry,
2-way sel pairs via tile_position row groups, flush after mains, outputs
per-group on the GpSimd ring. See kernel_iter4.py for the newer variant."""

import numpy as np

import concourse.bacc as bacc
import concourse.tile as tile
from concourse import mybir
from concourse.bass_utils import run_bass_kernel_spmd

B, T, F = 8, 4096, 1024
P = 128
NBLK = T // P  # 32
FH = 512       # one PSUM bank of f32
NHALF = F // FH
CPG = 2        # blocks per pipeline stage

F16 = mybir.dt.float16
F32 = mybir.dt.float32
F32R = mybir.dt.float32r


def _build():
    nc = bacc.Bacc(None, target_bir_lowering=False)
    x_dram = nc.dram_tensor("x", [T, F], F16, kind="ExternalInput")
    out_dram = nc.dram_tensor("out", [T, F], F16, kind="ExternalOutput")

    lt_np = np.triu(np.ones((P, P), dtype=np.float16))  # lt[s,t]=1 for s<=t
    sel_np = np.zeros((64, P), dtype=np.float16)        # row-group selectors
    sel_np[31, :] = 1.0
    sel_np[63, :] = 1.0
    recip_np = np.ascontiguousarray(
        (1.0 / (np.arange(1, T + 1, dtype=np.float64))).astype(np.float32)
        .reshape(NBLK, P).T
    )  # [p, i] = 1/(i*128+p+1)
    lt_dram = nc.inline_tensor(lt_np, "lt_const")
    sel_dram = nc.inline_tensor(sel_np, "sel_const")
    recip_dram = nc.inline_tensor(recip_np, "recip_const")

    x_rot = x_dram.rearrange("(n p) f -> p n f", p=P)
    out_rot = out_dram.rearrange("(n p) f -> p n f", p=P)

    with tile.TileContext(nc) as tc:
        with (
            tc.tile_pool(name="const", bufs=1) as cpool,
            tc.tile_pool(name="xin", bufs=6) as xpool,
            tc.tile_pool(name="xout", bufs=3) as opool,
            tc.tile_pool(name="run", bufs=6) as rpool,
            tc.tile_pool(name="psum", bufs=4, space="PSUM") as ppool,
        ):
            lt = cpool.tile([P, P], F16)
            nc.gpsimd.dma_start(lt[:], lt_dram[:])
            sel = cpool.tile([64, P], F16)
            nc.gpsimd.dma_start(sel[:], sel_dram[:])
            recip = cpool.tile([P, NBLK], F32)
            nc.gpsimd.dma_start(recip[:], recip_dram[:])

            def flush(pend, last=False):
                psums, carries, pbase, pgsz = pend
                ot = opool.tile([P, CPG, F], F16, tag="ot")
                for c in range(pgsz):
                    if carries[c] is not None:
                        for h in range(NHALF):
                            hs = slice(h * FH, (h + 1) * FH)
                            rs = slice(32 * h, 32 * h + 32)
                            nc.tensor.matmul(
                                psums[c][:, hs], sel[rs, :], carries[c][rs, :],
                                start=False, stop=True,
                                tile_position=(32 * h, 0),
                            )
                if last:
                    for c in range(pgsz):
                        i = pbase + c
                        nc.scalar.activation(
                            ot[:, c, :], psums[c][:],
                            mybir.ActivationFunctionType.Identity,
                            scale=recip[:, i : i + 1],
                        )
                        nc.gpsimd.dma_start(
                            out_rot[:, i : i + 1, :], ot[:, c : c + 1, :]
                        )
                else:
                    for c in range(pgsz):
                        i = pbase + c
                        nc.scalar.activation(
                            ot[:, c, :], psums[c][:],
                            mybir.ActivationFunctionType.Identity,
                            scale=recip[:, i : i + 1],
                        )
                    nc.gpsimd.dma_start(
                        out_rot[:, pbase : pbase + pgsz, :], ot[:, 0:pgsz, :]
                    )

            carry = None  # [64, FH] f32r split rows, see docstring
            pend = None
            base = 0
            for g in range(NBLK // CPG):
                if g == 0:
                    xt = xpool.tile([P, CPG, F], F16, tag="xt")
                    for c in range(CPG):
                        nc.sync.dma_start(
                            xt[:, c : c + 1, :], x_rot[:, c : c + 1, :]
                        )
                else:
                    xt = xpool.tile([P, CPG, F], F16, tag="xt")
                    nc.sync.dma_start(xt[:], x_rot[:, base : base + CPG, :])

                psums = []
                carries = []
                for c in range(CPG):
                    i = base + c
                    ps = ppool.tile([P, F], F32)
                    psums.append(ps)
                    carries.append(carry)
                    for h in range(NHALF):
                        hs = slice(h * FH, (h + 1) * FH)
                        nc.tensor.matmul(
                            ps[:, hs], lt[:], xt[:, c, hs],
                            start=True, stop=(i == 0),
                        )
                    if i < NBLK - 1:
                        new_carry = rpool.tile([64, FH], F16)
                        for h in range(NHALF):
                            hs = slice(h * FH, (h + 1) * FH)
                            rs = slice(32 * h, 32 * h + 32)
                            if carry is None:
                                nc.vector.tensor_copy(
                                    new_carry[rs, :], ps[96:P, hs]
                                )
                            else:
                                nc.vector.tensor_tensor(
                                    new_carry[rs, :],
                                    carry[rs, :],
                                    ps[96:P, hs],
                                    mybir.AluOpType.add,
                                )
                        carry = new_carry

                if pend is not None:
                    flush(pend)
                pend = (psums, carries, base, CPG)
                base += CPG

            flush(pend, last=True)

    nc.compile()
    return nc


_NC_CACHE = None
last_results = None  # BassKernelResults of the most recent run (for test harness)


def kernel(inputs: np.ndarray) -> np.ndarray:
    global _NC_CACHE, last_results
    if _NC_CACHE is None:
        _NC_CACHE = _build()
    nc = _NC_CACHE
    x = np.asarray(inputs)
    assert x.shape == (B, T, F), x.shape
    x16 = np.ascontiguousarray(x.astype(np.float16))
    in_maps = [{"x": x16[b]} for b in range(B)]
    res = run_bass_kernel_spmd(nc, in_maps, core_ids=list(range(B)))
    last_results = res
    return np.stack([r["out"] for r in res.results], axis=0).astype(np.float32)
